# revision 27
# baseline (speedup 1.0000x reference)
"""BiMamba layer Trainium2 kernel (8 NeuronCores, SPMD).

Sharding: 4 batch-groups x 2 d_inner-halves. Core (g, h) handles the 3
(b*f) scan units of batch g for d_inner channels [96h, 96h+96), both scan
directions. Each core emits a partial out-projection; the host sums the
two halves per batch and adds out_proj_b.

Per-core pipeline (per unit):
  conv+in_proj fused (PE, f32r, host-precomputed shifted matrices M_k)
  -> silu (ACT) -> x_proj / dt_proj (PE) -> softplus via Ln(Exp(p)+1) (ACT)
  -> delta replication to (d,n) rows (PE selection matmul -> PSUM)
  -> dA = Exp(A * delta) with per-partition A scale (ACT, bf16)
  -> dBu = du_rep * B_rep (DVE TT bf16 2x; du_rep via DMA broadcast)
  -> h = tensor_tensor_scan (DVE, fwd + time-reversed-AP bwd)
  -> hC (DVE TT bf16) -> n-reduction (PE selection matmul, + 2*u*D term)
  -> out_proj partial (PE f32r) -> HBM
"""
from contextlib import ExitStack

import numpy as np

import concourse.bass as bass
import concourse.tile as tile
from concourse import bacc, mybir
from concourse.bass_utils import run_bass_kernel_spmd

F32 = mybir.dt.float32
FR = mybir.dt.float32r
BF = mybir.dt.bfloat16
AF = mybir.ActivationFunctionType
OP = mybir.AluOpType

B, SEQ, DIM = 4, 6144, 384
L = 2048                  # per-unit sequence length
NU = 3                    # units per core
DIN, DH, NST, DTR = 192, 96, 16, 24
NK = 12                   # (DH*NST)//128 row-tiles
LC = 512                  # psum column chunk
NLC = L // LC
NCORES = 8

_NC_CACHE = {}


def _build(ab_same: bool, debug: bool = False):
    nc = bacc.Bacc("TRN2", target_bir_lowering=False, debug=False)

    def din(name, shape, dt=F32):
        return nc.dram_tensor(name, list(shape), dt, kind="ExternalInput").ap()

    xtp_d = din("xtp", (NU, 3, 128, L + 2))
    wm_d = din("wm", (9, 128, DIN))
    wmu_d = din("wmu", (9, 128, DH))
    efix_d = din("efix", (1, 2, DIN))
    efixu_d = din("efixu", (1, 2, DH))
    one_d = din("one", (1, 1))
    bsil_d = din("bsil", (128, 2))
    ubias_d = din("ubias", (DH, 1))
    wxp_d = din("wxp", (DIN, 56))
    wxpbc_d = din("wxpbc", (DIN, 256))
    wdt_d = din("wdt", (DTR, DH))
    bsp_d = din("bsp", (DH, 1))
    acol_d = din("acol", (NK * 128,))
    abcol_d = din("abcol", (NK * 128,))
    seli_d = din("seli", (DH, NK * 128))
    seln_d = din("seln", (4, 128, 32), BF)
    ddiag_d = din("ddiag", (DH, DH))
    wouty_d = din("wouty", (DH, DIM))
    woutz_d = din("woutz", (DH, DIM))
    out_d = nc.dram_tensor("out", [NU, L, DIM], F32, kind="ExternalOutput").ap()
    dbg = {}
    if debug:
        for name, shape, dt_ in [
                ("dbg_xc0", (128, L), F32), ("dbg_u", (DH, L), F32),
                ("dbg_dt", (DTR, L), F32), ("dbg_bc", (32, L), BF),
                ("dbg_delta", (DH, L), F32), ("dbg_du", (DH, L), BF),
                ("dbg_brep", (128, L), BF), ("dbg_daf", (128, L), BF),
                ("dbg_dbu", (128, L), BF), ("dbg_hf", (128, L), BF),
                ("dbg_hcb", (128, L), BF), ("dbg_y", (DH, L), F32)]:
            dbg[name] = nc.dram_tensor(name, list(shape), dt_,
                                       kind="ExternalOutput").ap()

    with tile.TileContext(nc) as tc, ExitStack() as ctx:
        cp = ctx.enter_context(tc.tile_pool(name="consts", bufs=1))
        px = ctx.enter_context(tc.tile_pool(name="px", bufs=1))
        pxc = ctx.enter_context(tc.tile_pool(name="pxc", bufs=1))
        pu = ctx.enter_context(tc.tile_pool(name="pu", bufs=2))
        psm = ctx.enter_context(tc.tile_pool(name="psm", bufs=1))
        pbig = ctx.enter_context(tc.tile_pool(name="pbig", bufs=2))
        pout = ctx.enter_context(tc.tile_pool(name="pout", bufs=2))
        ppa = ctx.enter_context(tc.tile_pool(name="ppa", bufs=2, space="PSUM"))
        ppd = ctx.enter_context(tc.tile_pool(name="ppd", bufs=2, space="PSUM"))
        ppy = ctx.enter_context(tc.tile_pool(name="ppy", bufs=4, space="PSUM"))

        # ---- constants ----
        wm_sb = cp.tile([128, 9, DIN], FR)
        nc.sync.dma_start(wm_sb[:], wm_d.transpose([1, 0, 2]).bitcast(FR))
        wmu_sb = cp.tile([128, 9, DH], FR)
        nc.sync.dma_start(wmu_sb[:], wmu_d.transpose([1, 0, 2]).bitcast(FR))
        efix_sb = cp.tile([1, 2, DIN], F32)
        nc.sync.dma_start(efix_sb[:], efix_d)
        efixu_sb = cp.tile([1, 2, DH], F32)
        nc.sync.dma_start(efixu_sb[:], efixu_d)
        one_sb = cp.tile([1, 1], F32)
        nc.sync.dma_start(one_sb[:], one_d)
        bsil_sb = cp.tile([128, 2], F32)
        nc.sync.dma_start(bsil_sb[:], bsil_d)
        ubias_sb = cp.tile([DH, 1], F32)
        nc.sync.dma_start(ubias_sb[:], ubias_d)
        wxp_sb = cp.tile([128, 56], FR)
        nc.sync.dma_start(wxp_sb[:], wxp_d[0:128, :].bitcast(FR))
        wxp2_sb = cp.tile([64, 56], FR)
        nc.sync.dma_start(wxp2_sb[:], wxp_d[128:192, :].bitcast(FR))
        wxpbc_sb = cp.tile([128, 256], FR)
        nc.sync.dma_start(wxpbc_sb[:], wxpbc_d[0:128, :].bitcast(FR))
        wxpbc2_sb = cp.tile([64, 256], FR)
        nc.sync.dma_start(wxpbc2_sb[:], wxpbc_d[128:192, :].bitcast(FR))
        wdt_sb = cp.tile([DTR, DH], FR)
        nc.sync.dma_start(wdt_sb[:], wdt_d.bitcast(FR))
        bsp_sb = cp.tile([DH, 1], F32)
        nc.sync.dma_start(bsp_sb[:], bsp_d)
        acol_sb = cp.tile([128, NK], F32)
        nc.sync.dma_start(acol_sb[:], acol_d.rearrange("(k p) -> p k", p=128))
        abcol_sb = cp.tile([128, NK], F32)
        nc.sync.dma_start(abcol_sb[:], abcol_d.rearrange("(k p) -> p k", p=128))
        seli_sb = cp.tile([DH, NK * 128], FR)
        nc.sync.dma_start(seli_sb[:], seli_d.bitcast(FR))
        seln_sb = cp.tile([128, 4, 32], BF)
        nc.sync.dma_start(seln_sb[:], seln_d.transpose([1, 0, 2]))
        ddiag_sb = cp.tile([DH, DH], FR)
        nc.sync.dma_start(ddiag_sb[:], ddiag_d.bitcast(FR))
        wouty_sb = cp.tile([DH, DIM], FR)
        nc.sync.dma_start(wouty_sb[:], wouty_d.bitcast(FR))
        woutz_sb = cp.tile([DH, DIM], FR)
        nc.sync.dma_start(woutz_sb[:], woutz_d.bitcast(FR))

        for u in range(NU):
            xt = px.tile([128, 3, L + 2], FR)
            nc.sync.dma_start(xt[:], xtp_d[u].transpose([1, 0, 2]).bitcast(FR))

            # ---- fused conv + in_proj ----
            xc0 = pxc.tile([128, L], FR)
            xc1 = pxc.tile([64, L], FR)
            u_sb = pu.tile([DH, L], FR)
            # three output groups: xc0 (ch 0:128), xc1 (ch 128:192), u (d-half)
            groups = [
                (wm_sb, 0, 128, xc0, bsil_sb[0:128, 0:1], efix_sb),
                (wm_sb, 128, 64, xc1, bsil_sb[0:64, 1:2], efix_sb),
                (wmu_sb, 0, DH, u_sb, ubias_sb[:], efixu_sb),
            ]
            for wsrc, c0, cw, dst, bias_ap, efx in groups:
                for lc in range(NLC):
                    ps = ppa.tile([128, LC], F32, tag="ppa", name="ps_conv")
                    mms = []
                    for s in range(3):
                        for kt in range(3):
                            mms.append((ps[0:cw, :],
                                        wsrc[:, s * 3 + kt, c0:c0 + cw],
                                        xt[:, kt, s + lc * LC:s + lc * LC + LC]))
                    if lc == 0:
                        mms.append((ps[0:cw, 0:1],
                                    efx[0:1, 0, c0:c0 + cw], one_sb[:]))
                    if lc == NLC - 1:
                        mms.append((ps[0:cw, LC - 1:LC],
                                    efx[0:1, 1, c0:c0 + cw], one_sb[:]))
                    for i, (o, lh, rh) in enumerate(mms):
                        nc.tensor.matmul(o, lh, rh, start=(i == 0),
                                         stop=(i == len(mms) - 1))
                    nc.scalar.activation(dst[:, lc * LC:(lc + 1) * LC],
                                         ps[0:cw, :], AF.Silu, bias=bias_ap)

            # ---- x_proj -> dt, and B/C already replicated to 128 rows ----
            dt_sb = psm.tile([DTR, L], FR)
            brep = psm.tile([128, L], BF)
            crep = psm.tile([128, L], BF)
            crep_rev = psm.tile([128, L], BF)
            xc0r = xc0[:, ::-1]
            xc1r = xc1[:, ::-1]
            for lc in range(NLC):
                sl = slice(lc * LC, (lc + 1) * LC)
                pdt = ppa.tile([128, LC], F32, tag="ppa", name="ps_dt")
                nc.tensor.matmul(pdt[0:DTR, :], wxp_sb[:, 0:DTR], xc0[:, sl],
                                 start=True, stop=False)
                nc.tensor.matmul(pdt[0:DTR, :], wxp2_sb[:, 0:DTR], xc1[:, sl],
                                 start=False, stop=True)
                nc.scalar.activation(dt_sb[:, sl], pdt[0:DTR, :], AF.Copy)
                pbr = ppa.tile([128, LC], F32, tag="ppa", name="ps_br")
                nc.tensor.matmul(pbr[:], wxpbc_sb[:, 0:128], xc0[:, sl],
                                 start=True, stop=False)
                nc.tensor.matmul(pbr[:], wxpbc2_sb[:, 0:128], xc1[:, sl],
                                 start=False, stop=True)
                nc.scalar.activation(brep[:, sl], pbr[:], AF.Copy)
                pcr = ppa.tile([128, LC], F32, tag="ppa", name="ps_cr")
                nc.tensor.matmul(pcr[:], wxpbc_sb[:, 128:256], xc0[:, sl],
                                 start=True, stop=False)
                nc.tensor.matmul(pcr[:], wxpbc2_sb[:, 128:256], xc1[:, sl],
                                 start=False, stop=True)
                nc.scalar.activation(crep[:, sl], pcr[:], AF.Copy)
                pcrr = ppa.tile([128, LC], F32, tag="ppa", name="ps_crr")
                nc.tensor.matmul(pcrr[:], wxpbc_sb[:, 128:256], xc0r[:, sl],
                                 start=True, stop=False)
                nc.tensor.matmul(pcrr[:], wxpbc2_sb[:, 128:256], xc1r[:, sl],
                                 start=False, stop=True)
                nc.scalar.activation(crep_rev[:, sl], pcrr[:], AF.Copy)

            if debug and u == 0:
                nc.sync.dma_start(dbg["dbg_xc0"], xc0[:].bitcast(F32))
                nc.sync.dma_start(dbg["dbg_u"], u_sb[:].bitcast(F32))
                nc.sync.dma_start(dbg["dbg_dt"], dt_sb[:].bitcast(F32))

            # ---- dt_proj + softplus ----
            delta_sb = psm.tile([DH, L], FR, bufs=2)
            esp = psm.tile([DH, L], F32)
            for lc in range(NLC):
                sl = slice(lc * LC, (lc + 1) * LC)
                pdp = ppa.tile([128, LC], F32, tag="ppa", name="ps_dp")
                nc.tensor.matmul(pdp[0:DH, :], wdt_sb[:], dt_sb[:, sl],
                                 start=True, stop=True)
                nc.scalar.activation(esp[:, sl], pdp[0:DH, :], AF.Exp,
                                     bias=bsp_sb[:])
            nc.scalar.activation(delta_sb[:], esp[:], AF.Ln, bias=1.0)

            # ---- du = delta * u (bf16) ----
            du_sb = psm.tile([DH, L], BF, bufs=2)
            nc.vector.tensor_tensor(du_sb[:], delta_sb[:], u_sb[:], OP.mult)


            if debug and u == 0:
                nc.sync.dma_start(dbg["dbg_delta"], delta_sb[:].bitcast(F32))
                nc.sync.dma_start(dbg["dbg_du"], du_sb[:])
            y_sb = psm.tile([DH, L], FR)

            # ---- main scan loop over row-tile groups ----
            for kg in range(3):
                pys = [ppy.tile([32, LC], F32, tag="pys", name="pys")
                       for _ in range(NLC)]
                for kk in range(4):
                    k = 4 * kg + kk
                    durep = pbig.tile([128, L], BF)
                    nc.gpsimd.dma_start(
                        durep[:],
                        du_sb[8 * k:8 * k + 8, :].unsqueeze(1)
                        .broadcast_to([8, 16, L]))
                    daf = pbig.tile([128, L], BF)
                    dab = None if ab_same else pbig.tile([128, L], BF)
                    for lc in range(NLC):
                        pd = ppd.tile([128, LC], F32)
                        nc.tensor.matmul(pd[:], seli_sb[:, 128 * k:128 * (k + 1)],
                                         delta_sb[:, lc * LC:(lc + 1) * LC],
                                         start=True, stop=True)
                        nc.scalar.activation(daf[:, lc * LC:(lc + 1) * LC],
                                             pd[:], AF.Exp,
                                             scale=acol_sb[:, k:k + 1])
                        if not ab_same:
                            nc.scalar.activation(dab[:, lc * LC:(lc + 1) * LC],
                                                 pd[:], AF.Exp,
                                                 scale=abcol_sb[:, k:k + 1])
                    dbu = pbig.tile([128, L], BF)
                    nc.vector.tensor_tensor(dbu[:], durep[:], brep[:], OP.mult)
                    hf = pbig.tile([128, L], BF)
                    nc.vector.tensor_tensor_scan(hf[:], daf[:], dbu[:], 0.0,
                                                 OP.mult, OP.add)
                    hb = pbig.tile([128, L], BF)
                    dab_src = daf if ab_same else dab
                    nc.vector.tensor_tensor_scan(hb[:], dab_src[:, ::-1],
                                                 dbu[:, ::-1], 0.0,
                                                 OP.mult, OP.add)
                    hcf = pbig.tile([128, L], BF)
                    nc.vector.tensor_tensor(hcf[:], hf[:], crep[:], OP.mult)
                    hcb = pbig.tile([128, L], BF)
                    nc.vector.tensor_tensor(hcb[:], hb[:], crep_rev[:],
                                            OP.mult)
                    if debug and u == 0 and k == 0:
                        nc.sync.dma_start(dbg["dbg_brep"], brep[:])
                        nc.sync.dma_start(dbg["dbg_daf"], daf[:])
                        nc.sync.dma_start(dbg["dbg_dbu"], dbu[:])
                        nc.sync.dma_start(dbg["dbg_hf"], hf[:])
                        nc.sync.dma_start(dbg["dbg_hcb"], hcb[:])
                    for lc in range(NLC):
                        sl = slice(lc * LC, (lc + 1) * LC)
                        nc.tensor.matmul(pys[lc][:], seln_sb[:, kk, :],
                                         hcf[:, sl], start=(kk == 0), stop=False)
                        nc.tensor.matmul(
                            pys[lc][:], seln_sb[:, kk, :],
                            hcb[:, ::-1][:, sl], start=False, stop=False)
                for lc in range(NLC):
                    sl = slice(lc * LC, (lc + 1) * LC)
                    nc.tensor.matmul(pys[lc][:], ddiag_sb[:, 32 * kg:32 * kg + 32],
                                     u_sb[:, sl], start=False, stop=True)
                    nc.scalar.activation(y_sb[32 * kg:32 * kg + 32, sl],
                                         pys[lc][:], AF.Copy)

            if debug and u == 0:
                nc.sync.dma_start(dbg["dbg_y"], y_sb[:].bitcast(F32))

            # ---- out_proj partial ----
            for tq in range(L // 256):
                osb = pout.tile([128, 2, DIM], F32)
                for j in range(2):
                    t8 = tq * 2 + j
                    sl = slice(t8 * 128, (t8 + 1) * 128)
                    po = ppa.tile([128, LC], F32, tag="ppa", name="ps_o")
                    nc.tensor.matmul(po[:, 0:DIM], y_sb[:, sl], wouty_sb[:],
                                     start=True, stop=False)
                    nc.tensor.matmul(po[:, 0:DIM], u_sb[:, sl], woutz_sb[:],
                                     start=False, stop=True)
                    nc.scalar.activation(osb[:, j, :], po[:, 0:DIM], AF.Copy)
                nc.sync.dma_start(
                    out_d[u, tq * 256:(tq + 1) * 256, :]
                    .rearrange("(j p) c -> p j c", p=128),
                    osb[:])

    nc.compile()
    return nc


def _get_nc(ab_same: bool):
    if ab_same not in _NC_CACHE:
        _NC_CACHE[ab_same] = _build(ab_same)
    return _NC_CACHE[ab_same]


def _prep_weights(h, in_proj_w, in_proj_b, conv_w, conv_b, A_log, Ab_log, D,
                  x_proj_w, dt_proj_w, dt_proj_b, out_proj_w):
    G = slice(96 * h, 96 * h + 96)
    f32 = np.float32
    W_in = in_proj_w.astype(f32)
    M = np.empty((3, DIN, DIM), f32)
    bconv = np.empty((3, DIN), f32)
    for k in range(3):
        M[k] = (conv_w[:, 0, k][:, None] * W_in[0::2, :]
                + conv_w[:, 1, k][:, None] * W_in[1::2, :])
        bconv[k] = (conv_w[:, 0, k] * in_proj_b[0::2]
                    + conv_w[:, 1, k] * in_proj_b[1::2])
    wm = np.empty((9, 128, DIN), f32)
    wmu = np.empty((9, 128, DH), f32)
    for s in range(3):
        for kt in range(3):
            wm[s * 3 + kt] = M[s][:, kt * 128:(kt + 1) * 128].T
            wmu[s * 3 + kt] = M[s][G, kt * 128:(kt + 1) * 128].T
    bias_int = bconv.sum(0) + conv_b
    efix = np.stack([-bconv[0], -bconv[2]])[None].astype(f32)
    efixu = efix[:, :, G].copy()
    bsil = np.zeros((128, 2), f32)
    bsil[:, 0] = bias_int[:128]
    bsil[0:64, 1] = bias_int[128:]
    A = (-np.exp(A_log)).astype(f32)
    Ab = (-np.exp(Ab_log)).astype(f32)
    seli = np.kron(np.eye(DH, dtype=f32), np.ones((1, NST), f32))
    seln = np.zeros((4, 128, 32), f32)
    for v in range(4):
        for r in range(128):
            seln[v, r, 8 * v + r // 16] = 1.0
    import ml_dtypes
    return dict(
        wm=wm,
        wmu=wmu,
        efix=efix,
        efixu=efixu,
        one=np.ones((1, 1), f32),
        bsil=bsil,
        ubias=bias_int[G].reshape(DH, 1).astype(f32),
        wxp=x_proj_w.T.astype(f32).copy(),
        wdt=dt_proj_w[G].T.astype(f32).copy(),
        bsp=dt_proj_b[G].reshape(DH, 1).astype(f32),
        acol=A[G].reshape(-1).copy(),
        abcol=Ab[G].reshape(-1).copy(),
        seli=seli,
        seln=seln.astype(ml_dtypes.bfloat16),
        ddiag=np.diag(2.0 * D[G]).astype(f32),
        wouty=out_proj_w[:, G].T.astype(f32).copy(),
        woutz=out_proj_w[:, 192 + 96 * h:192 + 96 * h + 96].T.astype(f32).copy(),
    )


def kernel(x, in_proj_w, in_proj_b, conv_w, conv_b, A_log, Ab_log, D,
           x_proj_w, dt_proj_w, dt_proj_b, out_proj_w, out_proj_b):
    ab_same = bool(np.array_equal(A_log, Ab_log))
    x = np.asarray(x, np.float32)

    wargs = (in_proj_w, in_proj_b, conv_w, conv_b, A_log, Ab_log, D,
             x_proj_w, dt_proj_w, dt_proj_b, out_proj_w)
    weights = [_prep_weights(h, *[np.asarray(a, np.float32) for a in wargs])
               for h in range(2)]

    in_maps = []
    for core in range(NCORES):
        g, h = divmod(core, 2)
        xtp = np.zeros((NU, 3, 128, L + 2), np.float32)
        for u in range(NU):
            xs = x[g, u * L:(u + 1) * L, :]        # (L, 384)
            xT = np.ascontiguousarray(xs.T)        # (384, L)
            xtp[u, :, :, 1:L + 1] = xT.reshape(3, 128, L)
        m = dict(weights[h])
        m["xtp"] = xtp
        in_maps.append(m)

    nc_prog = _get_nc(ab_same)
    r = run_bass_kernel_spmd(nc_prog, in_maps, list(range(NCORES)))
    res = r.results

    out = np.empty((B, SEQ, DIM), np.float32)
    bo = np.asarray(out_proj_b, np.float32)
    for g in range(B):
        for u in range(NU):
            part = (res[2 * g]["out"][u] + res[2 * g + 1]["out"][u] + bo)
            out[g, u * L:(u + 1) * L, :] = part
    return out
